# revision 32
# baseline (speedup 1.0000x reference)
"""Trainium2 Bass kernel for nn_AnatomicalSpaceAttention_5952824672905.

Self-contained: kernel(**inputs) takes the FULL unsharded inputs, shards
across 8 NeuronCores (core c -> batch c//4, D-planes [8*(c%4), 8*(c%4)+8)),
runs one SPMD Bass graph (no collectives -- cross-attention rows are
independent), and gathers the full [B, C, D, H, W] f32 output.

Structure (per core, 8192 query rows x 8 heads x 256 keys):
  - q-projection emits 64-dim "rope features" per head directly:
    weight columns are host-reordered to [h_even: q(32)|q_sw(32),
    h_odd: q(32)|q_sw(32)] per 128-chunk, so a single DVE multiply by a
    fused cos/sin table produces rot64 = [q*cos | q_sw*sin]; scores
    contract K=64 against krot duplicated twice -- no DVE add needed.
  - flat quarter pipeline: scores(Q)+exp(Q) -> avden(Q-1), uniform
    across pair boundaries so ScalarE (exp) never bubbles.
"""

import sys

for _p in ('/opt/trn_rl_repo', '/root/.axon_site/_ro/trn_rl_repo'):
    if _p not in sys.path:
        sys.path.append(_p)

import numpy as np
import ml_dtypes

import concourse.bass as bass
import concourse.mybir as mybir
import concourse.tile as tile
from concourse import bacc

BF16 = mybir.dt.bfloat16
F32 = mybir.dt.float32
AF = mybir.ActivationFunctionType

NH, HD = 8, 32
C, S, TD = 256, 256, 512
D = H = W = 32
N_CORES = 8
DSL = D // 4            # 8 d-planes per core
ROWS = DSL * H * W      # 8192
PT = 256                # rows per pair
NPAIRS = ROWS // PT     # 32
GR = 2048               # rows per DMA group
NG = ROWS // GR         # 4
PPG = GR // PT          # 8 pairs per group
SCALE = float(HD) ** -0.5
BASE = 10000.0

bf16 = ml_dtypes.bfloat16

# wpack block offsets (each block is [128, 128] bf16). Order = DMA
# priority: text-side weights first (they gate the serial prologue chain).
M1_O, TX_O, M2_O, KW_O, KSW_O, QW_O, VW_O, OW_O, NBLK = \
    0, 8, 16, 24, 40, 56, 64, 72, 76


# ----------------------------------------------------------------- host prep

def _inv_freq(dim):
    return 1.0 / (BASE ** (np.arange(0, dim, 2, dtype=np.float64) / dim))


def rope_freqs_full():
    """[D, H, W, HD] f64 -- matches reference.rope3d_freqs."""
    zd = HD // 3
    yd = HD // 3
    xd = HD - zd - yd
    fz = np.arange(D, dtype=np.float64)[:, None] * _inv_freq(zd)
    fy = np.arange(H, dtype=np.float64)[:, None] * _inv_freq(yd)
    fx = np.arange(W, dtype=np.float64)[:, None] * _inv_freq(xd)
    ez = np.broadcast_to(np.concatenate([fz, fz], -1)[:, None, None, :], (D, H, W, zd))
    ey = np.broadcast_to(np.concatenate([fy, fy], -1)[None, :, None, :], (D, H, W, yd))
    ex = np.broadcast_to(np.concatenate([fx, fx], -1)[None, None, :, :], (D, H, W, xd))
    return np.concatenate([ez, ey, ex], axis=-1)  # [D,H,W,HD]


def swap_w(wm):
    """Column-permute+negate so x @ w_sw == rotate_half(x @ w) per 32-head-dim."""
    w = np.asarray(wm)
    out = np.empty_like(w)
    for h in range(NH):
        blk = w[:, h * HD:(h + 1) * HD]
        out[:, h * HD:h * HD + 16] = -blk[:, 16:32]
        out[:, h * HD + 16:(h + 1) * HD] = blk[:, 0:16]
    return out


def pack64(wa, wb):
    """[K,256] x2 -> [K,512]; col ch*128 + a*64 + d*32 + j takes
    (wa if d==0 else wb)[:, 32*(2ch+a)+j]."""
    K = wa.shape[0]
    out = np.empty((K, 512), np.float64)
    o = out.reshape(K, 4, 2, 2, 32)
    A = wa.reshape(K, NH, 32)
    B = wb.reshape(K, NH, 32)
    for ch in range(4):
        for a in range(2):
            o[:, ch, a, 0, :] = A[:, 2 * ch + a, :]
            o[:, ch, a, 1, :] = B[:, 2 * ch + a, :]
    return out


def host_prep(inputs):
    fv = np.asarray(inputs['fused_visual'], dtype=np.float32)   # [B,C,D,H,W]
    te = np.asarray(inputs['text_embedding'], dtype=np.float32)  # [B,S,TD]
    q_w = np.asarray(inputs['q_w'], dtype=np.float64)
    k_w = np.asarray(inputs['k_w'], dtype=np.float64)
    v_w = np.asarray(inputs['v_w'], dtype=np.float64)
    o_w = np.asarray(inputs['o_w'], dtype=np.float64)
    m1_w = np.asarray(inputs['m1_w'], dtype=np.float64)
    m2_w = np.asarray(inputs['m2_w'], dtype=np.float64)

    qw64 = pack64(q_w * SCALE, swap_w(q_w) * SCALE)   # [256, 512]
    kw64 = pack64(k_w, k_w)                            # [512, 512]
    ksw64 = pack64(swap_w(k_w), swap_w(k_w))           # [512, 512]
    m2w64 = pack64(m2_w, m2_w)                         # [256, 512]

    freqs = rope_freqs_full()
    cosf = np.cos(freqs)
    sinf = np.sin(freqs)

    blocks = []

    def add64(w):
        for kc in range(w.shape[0] // 128):
            for ch in range(4):
                blocks.append(w[kc * 128:(kc + 1) * 128, ch * 128:(ch + 1) * 128])

    def add256(w):
        for kc in range(w.shape[0] // 128):
            blocks.append(w[kc * 128:(kc + 1) * 128, 0:128])
            blocks.append(w[kc * 128:(kc + 1) * 128, 128:256])

    in_maps = []
    for core in range(N_CORES):
        b = core // 4
        g4 = core % 4
        dsl = slice(g4 * DSL, (g4 + 1) * DSL)

        fv_sh = np.ascontiguousarray(
            fv[b, :, dsl].reshape(C, ROWS)).astype(bf16)

        cT = cosf[dsl].reshape(ROWS, HD).T   # [32, ROWS]
        sT = sinf[dsl].reshape(ROWS, HD).T
        cs64 = np.ascontiguousarray(
            np.concatenate([cT, sT, cT, sT], axis=0)).astype(bf16)  # [128, ROWS]

        textT = te[b].T.astype(np.float64)   # [TD, S]
        blocks.clear()
        add256(m1_w)
        add256(textT)
        add64(m2w64)
        add64(kw64)
        add64(ksw64)
        add64(qw64)
        add256(v_w)
        add256(o_w)
        wblob = np.ascontiguousarray(
            np.concatenate(blocks, axis=1)).astype(bf16)  # [128, 76*128]

        in_maps.append({'fv': fv_sh, 'cs': cs64, 'wpack': wblob})
    return in_maps


def gather_out(results):
    """Per-core [C, ROWS] bf16 -> full [B, C, D, H, W] f32."""
    B = 2
    out = np.empty((B, C, D, H, W), dtype=np.float32)
    for core in range(N_CORES):
        b = core // 4
        g4 = core % 4
        out[b, :, g4 * DSL:(g4 + 1) * DSL] = (
            np.asarray(results[core]['out']).astype(np.float32)
            .reshape(C, DSL, H, W))
    return out


# ------------------------------------------------------------------- builder

def build_nc():
    nc = bacc.Bacc("TRN2", target_bir_lowering=False, debug=False)

    fv_d = nc.dram_tensor("fv", [C, ROWS], BF16, kind="ExternalInput")
    cs_d = nc.dram_tensor("cs", [128, ROWS], BF16, kind="ExternalInput")
    wpack_d = nc.dram_tensor("wpack", [128, NBLK * 128], BF16,
                             kind="ExternalInput")
    out_d = nc.dram_tensor("out", [C, ROWS], BF16, kind="ExternalOutput")

    with tile.TileContext(nc) as tc:
        _graph(tc, nc, fv_d, cs_d, wpack_d, out_d)

    nc.compile()
    return nc


def _graph(tc, nc, fv_d, cs_d, wpack_d, out_d):
    from contextlib import ExitStack
    ctx = ExitStack()
    with ctx:
        const = ctx.enter_context(tc.tile_pool(name="const", bufs=1))
        io = ctx.enter_context(tc.tile_pool(name="io", bufs=2))
        work = ctx.enter_context(tc.tile_pool(name="work", bufs=2))
        expp = ctx.enter_context(tc.tile_pool(name="expp", bufs=6))
        pq = ctx.enter_context(tc.tile_pool(name="pq", bufs=1, space="PSUM"))
        ps = ctx.enter_context(tc.tile_pool(name="ps", bufs=1, space="PSUM"))
        pa = ctx.enter_context(tc.tile_pool(name="pa", bufs=1, space="PSUM"))
        po = ctx.enter_context(tc.tile_pool(name="po", bufs=1, space="PSUM"))

        # ---------- PE warm-up burst (opens HAM clock gate) ----------
        wub = const.tile([1, 512], BF16)
        nc.vector.memset(wub, 0.0)
        wu = ps.tile([128, 2, 2, PT], F32, tag="sp0", name="wu")
        wuf = wu.rearrange("p a b r -> p (a b r)")
        for _ in range(7):
            nc.tensor.matmul(out=wuf[:, 0:512], lhsT=wub[0:1, 0:128],
                             rhs=wub, start=True, stop=True)

        # ---------- DMAs ----------
        # Split by prologue criticality: text-side weights gate the serial
        # krot chain; qw/fv/cs gate pair-0 q-proj; vw/ow are needed later.
        wpack_sb = const.tile([128, NBLK, 128], BF16)

        def wdma(b0, b1):
            nc.sync.dma_start(
                out=wpack_sb[:, b0:b1].rearrange("p a b -> p (a b)"),
                in_=wpack_d[:, b0 * 128:b1 * 128])

        wdma(M1_O, M2_O)           # m1 + textT
        wdma(M2_O, M2_O + 8)       # m2w64
        wdma(KW_O, KSW_O + 16)     # kw64 + ksw64
        wdma(QW_O, QW_O + 8)       # qw64

        cs_sb = const.tile([128, ROWS], BF16)
        fvst_tiles = {}

        def load_group(gi):
            t = io.tile([128, 2, GR], BF16, tag="fvst", name="fvst")
            for kc in range(2):
                nc.sync.dma_start(
                    out=t[:, kc, :],
                    in_=fv_d[kc * 128:(kc + 1) * 128, gi * GR:(gi + 1) * GR])
            nc.sync.dma_start(out=cs_sb[:, gi * GR:(gi + 1) * GR],
                              in_=cs_d[:, gi * GR:(gi + 1) * GR])
            fvst_tiles[gi] = t

        load_group(0)
        wdma(VW_O, NBLK)           # vw + ow

        # ---------- constants ----------
        ones_sb = const.tile([128, HD], BF16)
        nc.vector.memset(ones_sb, 1.0)
        pi2_sb = const.tile([128, 1], F32)
        nc.vector.memset(pi2_sb, float(np.pi / 2))

        def wchunk(idx):
            return wpack_sb[:, idx, :]

        def qwv(kc, ch):
            return wchunk(QW_O + kc * 4 + ch)

        def kwv(kc, ch):
            return wchunk(KW_O + kc * 4 + ch)

        def kswv(kc, ch):
            return wchunk(KSW_O + kc * 4 + ch)

        def m2v(kc, ch):
            return wchunk(M2_O + kc * 4 + ch)

        vwv = wpack_sb[:, VW_O:VW_O + 8].rearrange("p (a b) f -> p a (b f)", b=2)
        m1v = wpack_sb[:, M1_O:M1_O + 8].rearrange("p (a b) f -> p a (b f)", b=2)
        owv = wpack_sb[:, OW_O:OW_O + 4].rearrange("p (a b) f -> p a (b f)", b=2)
        txv = wpack_sb[:, TX_O:TX_O + 8].rearrange("p (a b) f -> p a (b f)", b=2)

        # ---------- text side (once) ----------
        # h1 = gelu(text @ m1_w): [128, 2(mc), S] bf16
        h1_sb = const.tile([128, 2, S], BF16)
        for mc in range(2):
            pool = pq if mc == 0 else po
            h1ps = pool.tile([128, 2, PT], F32,
                             tag="qp" if mc == 0 else "op", name="h1ps")
            pv = h1ps.rearrange("p a r -> p (a r)")[:, 0:S]
            for kc in range(4):
                nc.tensor.matmul(out=pv,
                                 lhsT=m1v[:, kc, mc * 128:(mc + 1) * 128],
                                 rhs=txv[:, kc, :],
                                 start=(kc == 0), stop=(kc == 3))
            nc.scalar.activation(out=h1_sb[:, mc, :], in_=pv, func=AF.Gelu)

        # phase64 = h1 @ m2w64 (dup'd cols) -> cos/sin
        ph64 = pa.tile([128, 4, PT], F32, tag="avd", name="ph64")
        for ch in range(4):
            for kc in range(2):
                nc.tensor.matmul(out=ph64[:, ch, :],
                                 lhsT=m2v(kc, ch),
                                 rhs=h1_sb[:, kc, :],
                                 start=(kc == 0), stop=(kc == 1))
        csph = const.tile([128, 4, 2, S], F32)
        nc.scalar.activation(out=csph[:, :, 0, :], in_=ph64, func=AF.Sin,
                             bias=pi2_sb)
        nc.scalar.activation(out=csph[:, :, 1, :], in_=ph64, func=AF.Sin)

        # k / k_sw projections with dup'd column layout; issued after the
        # pair-0 q-rope fills so those aren't queued behind 32 cold matmuls
        kproj = {}

        def build_kproj():
            kk = ps.tile([128, 2, 2, PT], F32, tag="sp0", name="kk")
            kkv = kk.rearrange("p a b r -> p (a b) r")
            ks = ps.tile([128, 2, 2, PT], F32, tag="sp1", name="ks")
            ksv = ks.rearrange("p a b r -> p (a b) r")
            for ch in (0, 3, 1, 2):
                for kc in range(4):
                    nc.tensor.matmul(out=kkv[:, ch, :], lhsT=kwv(kc, ch),
                                     rhs=txv[:, kc, :],
                                     start=(kc == 0), stop=(kc == 3))
                for kc in range(4):
                    nc.tensor.matmul(out=ksv[:, ch, :], lhsT=kswv(kc, ch),
                                     rhs=txv[:, kc, :],
                                     start=(kc == 0), stop=(kc == 3))
            kproj['kkv'] = kkv
            kproj['ksv'] = ksv

        tmpa = const.tile([128, 4, S], F32)
        tmpb = const.tile([128, 4, S], F32)
        krot_sb = const.tile([128, 4, S], BF16)
        v_sb = const.tile([128, 2, C], BF16)

        # ---------- main loop helpers ----------

        def qrope_fill(pi, cp, rot_t):
            """q-proj + rope for pair pi, chunk-pair cp -> rot_t[:, 2cp:2cp+2]."""
            gi = pi // PPG
            p0 = (pi % PPG) * PT
            fvst = fvst_tiles[gi]
            qp = pq.tile([128, 2, PT], F32, tag="qp", name="qp")
            for ch2 in range(2):
                ch = 2 * cp + ch2
                for kc in range(2):
                    nc.tensor.matmul(
                        out=qp[:, ch2, :], lhsT=qwv(kc, ch),
                        rhs=fvst[:, kc, p0:p0 + PT],
                        start=(kc == 0), stop=(kc == 1))
            csap = (cs_sb[:, pi * PT:(pi + 1) * PT]
                    .unsqueeze(1).broadcast_to([128, 2, PT]))
            nc.vector.tensor_mul(rot_t[:, 2 * cp:2 * cp + 2, :], qp, csap)

        # Quarter schedule within a pair. Positions pair up (0,1) and (2,3);
        # paired quarters have different `half` (disjoint PE col strips ->
        # 4-way tile concurrency) AND different `g` (disjoint PSUM banks in
        # the g-interleaved avd layout [av-g0, den-g0, av-g1, den-g1]).
        QSEQ = (0, 3, 2, 1)

        def scores(Qp, rot_t):
            p = Qp % 4
            qd = QSEQ[p]
            half, g = qd // 2, qd % 2
            ch = 2 * g + half
            sp = ps.tile([128, 2, 2, PT], F32, tag=f"sp{p % 2}",
                         name="sp", uniquify=True)
            ex = expp.tile([128, 2, 2, PT], BF16, tag="ex",
                           name="ex", uniquify=True)
            for c in range(2):
                for hp2 in range(2):
                    nc.tensor.matmul(
                        out=sp[:, hp2, c, :],
                        lhsT=krot_sb[64 * hp2:64 * hp2 + 64, ch,
                                     c * 128:(c + 1) * 128],
                        rhs=rot_t[64 * hp2:64 * hp2 + 64, ch, :],
                        start=True, stop=True,
                        tile_position=(64 * hp2, 0))
            nc.scalar.activation(out=ex.rearrange("p a b r -> p (a b r)"),
                                 in_=sp.rearrange("p a b r -> p (a b r)"),
                                 func=AF.Exp)
            return ex

        def avden2(avd, exa, qda, exb, qdb):
            """Interleave two quarters' av+den matmuls. The quarters use
            disjoint col strips and disjoint PSUM banks, so the matmuls run
            4-way concurrent while same-bank accumulation groups stay
            sequential."""
            def av_mm(ex, qd, hp2, c):
                half, g = qd // 2, qd % 2
                hp = 2 * half + hp2
                h = 4 * g + hp
                nc.tensor.matmul(out=avd[32 * hp:32 * hp + 32, 2 * g, :],
                                 lhsT=v_sb[:, c, 32 * h:32 * h + 32],
                                 rhs=ex[:, hp2, c, :],
                                 start=(c == 0), stop=(c == 1),
                                 tile_position=(0, 32 * hp))

            def den_mm(ex, qd, hp2, c):
                half, g = qd // 2, qd % 2
                hp = 2 * half + hp2
                nc.tensor.matmul(out=avd[32 * hp:32 * hp + 32, 2 * g + 1, :],
                                 lhsT=ones_sb,
                                 rhs=ex[:, hp2, c, :],
                                 start=(c == 0), stop=(c == 1),
                                 tile_position=(0, 32 * hp))

            for hp2 in range(2):
                for c in range(2):
                    av_mm(exa, qda, hp2, c)
                    av_mm(exb, qdb, hp2, c)
            for hp2 in range(2):
                for c in range(2):
                    den_mm(exa, qda, hp2, c)
                    den_mm(exb, qdb, hp2, c)

        st = {'outst': None}
        HG = 4 * PT  # rows per output-DMA chunk (half group)

        def oproj(pi, adv):
            gi2 = pi // PPG
            if pi % PPG == 0:
                st['outst'] = io.tile([128, 2, GR], BF16, tag="outst",
                                      name="outst")
            outst = st['outst']
            op = po.tile([128, 2, PT], F32, tag="op", name="op")
            for mc in range(2):
                for g in range(2):
                    nc.tensor.matmul(
                        out=op[:, mc, :],
                        lhsT=owv[:, g, mc * 128:(mc + 1) * 128],
                        rhs=adv[:, g, :], start=(g == 0), stop=(g == 1))
            p0 = (pi % PPG) * PT
            nc.vector.tensor_copy(outst[:, :, p0:p0 + PT], op)
            if gi2 == NG - 1:
                for mc in range(2):
                    nc.sync.dma_start(
                        out=out_d[mc * 128:(mc + 1) * 128,
                                  gi2 * GR + p0:gi2 * GR + p0 + PT],
                        in_=outst[:, mc, p0:p0 + PT])
            elif pi % 4 == 3:
                h0 = (pi % PPG - 3) * PT
                for mc in range(2):
                    nc.sync.dma_start(
                        out=out_d[mc * 128:(mc + 1) * 128,
                                  gi2 * GR + h0:gi2 * GR + h0 + HG],
                        in_=outst[:, mc, h0:h0 + HG])

        def recip_div(avd):
            avg = avd.rearrange("p (g s) r -> p g s r", s=2)
            rbc = work.tile([128, 2, PT], F32, tag="rbc", name="rbc")
            nc.vector.reciprocal_approx_fast(rbc, avg[:, :, 1, :])
            adv = work.tile([128, 2, PT], BF16, tag="adiv", name="adiv")
            nc.vector.tensor_mul(adv, avg[:, :, 0, :], rbc)
            return adv

        # krot combine per chunk, in main-loop consumption order.
        def krot_combine():
            kkv, ksv = kproj['kkv'], kproj['ksv']
            for ch in (0, 3, 1, 2):
                nc.vector.tensor_mul(tmpa[:, ch, :], kkv[:, ch, :],
                                     csph[:, ch, 0, :])
                nc.vector.tensor_mul(tmpb[:, ch, :], ksv[:, ch, :],
                                     csph[:, ch, 1, :])
                nc.vector.tensor_add(krot_sb[:, ch, :], tmpa[:, ch, :],
                                     tmpb[:, ch, :])

        # v: [S-chunk, C] bf16 -- first consumed at slot 3; issued after the
        # first scores so its (cold) matmuls don't delay the first exp.
        def build_v():
            for sc in range(2):
                vps = po.tile([128, 2, PT], F32, tag="op", name="vps")
                pv = vps.rearrange("p a r -> p (a r)")[:, 0:C]
                for kc in range(4):
                    nc.tensor.matmul(out=pv,
                                     lhsT=txv[:, kc, sc * 128:(sc + 1) * 128],
                                     rhs=vwv[:, kc, :],
                                     start=(kc == 0), stop=(kc == 3))
                nc.vector.tensor_copy(v_sb[:, sc, :], pv)

        # ---------- main loop: flat quarter pipeline, avden2 at lag 3 ----------

        ex_hist = {}
        avd_by_pair = {}
        adv_by_pair = {}
        rot_by_pair = {}

        def avden_pair(Qa):
            """avden for positions (Qa, Qa+1); Qa % 4 in {0, 2}."""
            P2, pa_pos = divmod(Qa, 4)
            if pa_pos == 0:
                avd_by_pair[P2] = pa.tile([128, 4, PT], F32, tag="avd",
                                          name="avd", uniquify=True)
            avden2(avd_by_pair[P2],
                   ex_hist.pop(Qa), QSEQ[pa_pos],
                   ex_hist.pop(Qa + 1), QSEQ[pa_pos + 1])
            if pa_pos == 2:
                adv_by_pair[P2] = recip_div(avd_by_pair.pop(P2))

        NQ = 4 * NPAIRS
        rot_by_pair[0] = work.tile([128, 4, PT], BF16, tag="rot", name="rot")
        qrope_fill(0, 0, rot_by_pair[0])
        qrope_fill(0, 1, rot_by_pair[0])
        build_kproj()
        krot_combine()

        for Q in range(NQ):
            P, p = divmod(Q, 4)
            gi = P // PPG
            ex_hist[Q] = scores(Q, rot_by_pair[P])
            if Q == 0:
                build_v()
            if p == 0 and P + 1 < NPAIRS:
                rot_by_pair[P + 1] = work.tile([128, 4, PT], BF16, tag="rot",
                                               name="rot", uniquify=True)
                qrope_fill(P + 1, 0, rot_by_pair[P + 1])
            if p == 1 and P + 1 < NPAIRS:
                qrope_fill(P + 1, 1, rot_by_pair[P + 1])
            if p % 2 == 0 and Q >= 2:
                avden_pair(Q - 2)
            if p == 1 and P - 1 in adv_by_pair:
                oproj(P - 1, adv_by_pair.pop(P - 1))
            if p == 3:
                rot_by_pair.pop(P)
                if P % PPG == 0 and gi + 1 < NG:
                    load_group(gi + 1)

        # ---------- tail ----------
        avden_pair(NQ - 2)
        oproj(NPAIRS - 1, adv_by_pair.pop(NPAIRS - 1))


_NC_CACHE = {}


def _get_nc():
    if 'nc' not in _NC_CACHE:
        _NC_CACHE['nc'] = build_nc()
    return _NC_CACHE['nc']


def _run(inputs, trace=False):
    from concourse.bass_utils import run_bass_kernel_spmd
    nc = _get_nc()
    in_maps = host_prep(inputs)
    res = run_bass_kernel_spmd(nc, in_maps, core_ids=list(range(N_CORES)),
                               trace=trace)
    return gather_out(res.results), res


def kernel(**inputs):
    out, _ = _run(inputs, trace=False)
    return out


# revision 33
# speedup vs baseline: 1.0864x; 1.0864x over previous
"""Trainium2 Bass kernel for nn_AnatomicalSpaceAttention_5952824672905.

Self-contained: kernel(**inputs) takes the FULL unsharded inputs, shards
across 8 NeuronCores (core c -> batch c//4, D-planes [8*(c%4), 8*(c%4)+8)),
runs one SPMD Bass graph (no collectives -- cross-attention rows are
independent), and gathers the full [B, C, D, H, W] f32 output.

Structure (per core, 8192 query rows x 8 heads x 256 keys):
  - q-projection emits 64-dim "rope features" per head directly:
    weight columns are host-reordered to [h_even: q(32)|q_sw(32),
    h_odd: q(32)|q_sw(32)] per 128-chunk, so a single DVE multiply by a
    fused cos/sin table produces rot64 = [q*cos | q_sw*sin]; scores
    contract K=64 against krot duplicated twice -- no DVE add needed.
  - flat quarter pipeline: scores(Q)+exp(Q) -> avden(Q-1), uniform
    across pair boundaries so ScalarE (exp) never bubbles.
"""

import sys

for _p in ('/opt/trn_rl_repo', '/root/.axon_site/_ro/trn_rl_repo'):
    if _p not in sys.path:
        sys.path.append(_p)

import numpy as np
import ml_dtypes

import concourse.bass as bass
import concourse.mybir as mybir
import concourse.tile as tile
from concourse import bacc

BF16 = mybir.dt.bfloat16
F32 = mybir.dt.float32
AF = mybir.ActivationFunctionType

NH, HD = 8, 32
C, S, TD = 256, 256, 512
D = H = W = 32
N_CORES = 8
DSL = D // 4            # 8 d-planes per core
ROWS = DSL * H * W      # 8192
PT = 256                # rows per pair
NPAIRS = ROWS // PT     # 32
GR = 2048               # rows per DMA group
NG = ROWS // GR         # 4
PPG = GR // PT          # 8 pairs per group
SCALE = float(HD) ** -0.5
BASE = 10000.0

bf16 = ml_dtypes.bfloat16

# wpack block offsets (each block is [128, 128] bf16). Order = DMA
# priority: text-side weights first (they gate the serial prologue chain).
M1_O, TX_O, M2_O, KW_O, KSW_O, QW_O, VW_O, OW_O, NBLK = \
    0, 8, 16, 24, 40, 56, 64, 72, 76


# ----------------------------------------------------------------- host prep

def _inv_freq(dim):
    return 1.0 / (BASE ** (np.arange(0, dim, 2, dtype=np.float64) / dim))


def rope_freqs_full():
    """[D, H, W, HD] f64 -- matches reference.rope3d_freqs."""
    zd = HD // 3
    yd = HD // 3
    xd = HD - zd - yd
    fz = np.arange(D, dtype=np.float64)[:, None] * _inv_freq(zd)
    fy = np.arange(H, dtype=np.float64)[:, None] * _inv_freq(yd)
    fx = np.arange(W, dtype=np.float64)[:, None] * _inv_freq(xd)
    ez = np.broadcast_to(np.concatenate([fz, fz], -1)[:, None, None, :], (D, H, W, zd))
    ey = np.broadcast_to(np.concatenate([fy, fy], -1)[None, :, None, :], (D, H, W, yd))
    ex = np.broadcast_to(np.concatenate([fx, fx], -1)[None, None, :, :], (D, H, W, xd))
    return np.concatenate([ez, ey, ex], axis=-1)  # [D,H,W,HD]


def swap_w(wm):
    """Column-permute+negate so x @ w_sw == rotate_half(x @ w) per 32-head-dim."""
    w = np.asarray(wm)
    out = np.empty_like(w)
    for h in range(NH):
        blk = w[:, h * HD:(h + 1) * HD]
        out[:, h * HD:h * HD + 16] = -blk[:, 16:32]
        out[:, h * HD + 16:(h + 1) * HD] = blk[:, 0:16]
    return out


def pack64(wa, wb):
    """[K,256] x2 -> [K,512]; col ch*128 + a*64 + d*32 + j takes
    (wa if d==0 else wb)[:, 32*(2ch+a)+j]."""
    K = wa.shape[0]
    out = np.empty((K, 512), np.float64)
    o = out.reshape(K, 4, 2, 2, 32)
    A = wa.reshape(K, NH, 32)
    B = wb.reshape(K, NH, 32)
    for ch in range(4):
        for a in range(2):
            o[:, ch, a, 0, :] = A[:, 2 * ch + a, :]
            o[:, ch, a, 1, :] = B[:, 2 * ch + a, :]
    return out


def host_prep(inputs):
    fv = np.asarray(inputs['fused_visual'], dtype=np.float32)   # [B,C,D,H,W]
    te = np.asarray(inputs['text_embedding'], dtype=np.float32)  # [B,S,TD]
    q_w = np.asarray(inputs['q_w'], dtype=np.float64)
    k_w = np.asarray(inputs['k_w'], dtype=np.float64)
    v_w = np.asarray(inputs['v_w'], dtype=np.float64)
    o_w = np.asarray(inputs['o_w'], dtype=np.float64)
    m1_w = np.asarray(inputs['m1_w'], dtype=np.float64)
    m2_w = np.asarray(inputs['m2_w'], dtype=np.float64)

    qw64 = pack64(q_w * SCALE, swap_w(q_w) * SCALE)   # [256, 512]
    kw64 = pack64(k_w, k_w)                            # [512, 512]
    ksw64 = pack64(swap_w(k_w), swap_w(k_w))           # [512, 512]
    m2w64 = pack64(m2_w, m2_w)                         # [256, 512]

    freqs = rope_freqs_full()
    cosf = np.cos(freqs)
    sinf = np.sin(freqs)

    blocks = []

    def add64(w):
        for kc in range(w.shape[0] // 128):
            for ch in range(4):
                blocks.append(w[kc * 128:(kc + 1) * 128, ch * 128:(ch + 1) * 128])

    def add256(w):
        for kc in range(w.shape[0] // 128):
            blocks.append(w[kc * 128:(kc + 1) * 128, 0:128])
            blocks.append(w[kc * 128:(kc + 1) * 128, 128:256])

    in_maps = []
    for core in range(N_CORES):
        b = core // 4
        g4 = core % 4
        dsl = slice(g4 * DSL, (g4 + 1) * DSL)

        fv_sh = np.ascontiguousarray(
            fv[b, :, dsl].reshape(C, ROWS)).astype(bf16)

        cT = cosf[dsl].reshape(ROWS, HD).T   # [32, ROWS]
        sT = sinf[dsl].reshape(ROWS, HD).T
        cs64 = np.ascontiguousarray(
            np.concatenate([cT, sT, cT, sT], axis=0)).astype(bf16)  # [128, ROWS]

        textT = te[b].T.astype(np.float64)   # [TD, S]
        blocks.clear()
        add256(m1_w)
        add256(textT)
        add64(m2w64)
        add64(kw64)
        add64(ksw64)
        add64(qw64)
        add256(v_w)
        add256(o_w)
        wblob = np.ascontiguousarray(
            np.concatenate(blocks, axis=1)).astype(bf16)  # [128, 76*128]

        in_maps.append({'fv': fv_sh, 'cs': cs64, 'wpack': wblob})
    return in_maps


def gather_out(results):
    """Per-core [C, ROWS] bf16 -> full [B, C, D, H, W] f32."""
    B = 2
    out = np.empty((B, C, D, H, W), dtype=np.float32)
    for core in range(N_CORES):
        b = core // 4
        g4 = core % 4
        out[b, :, g4 * DSL:(g4 + 1) * DSL] = (
            np.asarray(results[core]['out']).astype(np.float32)
            .reshape(C, DSL, H, W))
    return out


# ------------------------------------------------------------------- builder

def build_nc():
    nc = bacc.Bacc("TRN2", target_bir_lowering=False, debug=False)

    fv_d = nc.dram_tensor("fv", [C, ROWS], BF16, kind="ExternalInput")
    cs_d = nc.dram_tensor("cs", [128, ROWS], BF16, kind="ExternalInput")
    wpack_d = nc.dram_tensor("wpack", [128, NBLK * 128], BF16,
                             kind="ExternalInput")
    out_d = nc.dram_tensor("out", [C, ROWS], BF16, kind="ExternalOutput")

    with tile.TileContext(nc) as tc:
        _graph(tc, nc, fv_d, cs_d, wpack_d, out_d)

    nc.compile()
    return nc


def _graph(tc, nc, fv_d, cs_d, wpack_d, out_d):
    from contextlib import ExitStack
    ctx = ExitStack()
    with ctx:
        const = ctx.enter_context(tc.tile_pool(name="const", bufs=1))
        io = ctx.enter_context(tc.tile_pool(name="io", bufs=2))
        work = ctx.enter_context(tc.tile_pool(name="work", bufs=2))
        expp = ctx.enter_context(tc.tile_pool(name="expp", bufs=6))
        pq = ctx.enter_context(tc.tile_pool(name="pq", bufs=1, space="PSUM"))
        ps = ctx.enter_context(tc.tile_pool(name="ps", bufs=1, space="PSUM"))
        pa = ctx.enter_context(tc.tile_pool(name="pa", bufs=1, space="PSUM"))
        po = ctx.enter_context(tc.tile_pool(name="po", bufs=1, space="PSUM"))

        # ---------- PE warm-up burst (opens HAM clock gate) ----------
        wub = const.tile([1, 512], BF16)
        nc.vector.memset(wub, 0.0)
        wu = ps.tile([128, 2, 2, PT], F32, tag="sp0", name="wu")
        wuf = wu.rearrange("p a b r -> p (a b r)")
        for _ in range(7):
            nc.tensor.matmul(out=wuf[:, 0:512], lhsT=wub[0:1, 0:128],
                             rhs=wub, start=True, stop=True)

        # ---------- DMAs ----------
        # Split by prologue criticality: text-side weights gate the serial
        # krot chain; qw/fv/cs gate pair-0 q-proj; vw/ow are needed later.
        wpack_sb = const.tile([128, NBLK, 128], BF16)

        def wdma(b0, b1):
            nc.sync.dma_start(
                out=wpack_sb[:, b0:b1].rearrange("p a b -> p (a b)"),
                in_=wpack_d[:, b0 * 128:b1 * 128])

        wdma(M1_O, M2_O)           # m1 + textT
        wdma(M2_O, M2_O + 8)       # m2w64
        wdma(KW_O, KSW_O + 16)     # kw64 + ksw64
        wdma(QW_O, QW_O + 8)       # qw64

        cs_sb = const.tile([128, ROWS], BF16)
        fvst_tiles = {}

        def load_group(gi):
            t = io.tile([128, 2, GR], BF16, tag="fvst", name="fvst")
            for kc in range(2):
                nc.sync.dma_start(
                    out=t[:, kc, :],
                    in_=fv_d[kc * 128:(kc + 1) * 128, gi * GR:(gi + 1) * GR])
            nc.sync.dma_start(out=cs_sb[:, gi * GR:(gi + 1) * GR],
                              in_=cs_d[:, gi * GR:(gi + 1) * GR])
            fvst_tiles[gi] = t

        load_group(0)
        wdma(VW_O, NBLK)           # vw + ow

        # ---------- constants ----------
        ones_sb = const.tile([128, HD], BF16)
        nc.vector.memset(ones_sb, 1.0)
        pi2_sb = const.tile([128, 1], F32)
        nc.vector.memset(pi2_sb, float(np.pi / 2))

        def wchunk(idx):
            return wpack_sb[:, idx, :]

        def qwv(kc, ch):
            return wchunk(QW_O + kc * 4 + ch)

        def kwv(kc, ch):
            return wchunk(KW_O + kc * 4 + ch)

        def kswv(kc, ch):
            return wchunk(KSW_O + kc * 4 + ch)

        def m2v(kc, ch):
            return wchunk(M2_O + kc * 4 + ch)

        vwv = wpack_sb[:, VW_O:VW_O + 8].rearrange("p (a b) f -> p a (b f)", b=2)
        m1v = wpack_sb[:, M1_O:M1_O + 8].rearrange("p (a b) f -> p a (b f)", b=2)
        owv = wpack_sb[:, OW_O:OW_O + 4].rearrange("p (a b) f -> p a (b f)", b=2)
        txv = wpack_sb[:, TX_O:TX_O + 8].rearrange("p (a b) f -> p a (b f)", b=2)

        # ---------- text side (once) ----------
        # h1 = gelu(text @ m1_w): [128, 2(mc), S] bf16
        h1_sb = const.tile([128, 2, S], BF16)
        for mc in range(2):
            pool = pq if mc == 0 else po
            h1ps = pool.tile([128, 2, PT], F32,
                             tag="qp" if mc == 0 else "op", name="h1ps")
            pv = h1ps.rearrange("p a r -> p (a r)")[:, 0:S]
            for kc in range(4):
                nc.tensor.matmul(out=pv,
                                 lhsT=m1v[:, kc, mc * 128:(mc + 1) * 128],
                                 rhs=txv[:, kc, :],
                                 start=(kc == 0), stop=(kc == 3))
            nc.scalar.activation(out=h1_sb[:, mc, :], in_=pv, func=AF.Gelu)

        # phase64 = h1 @ m2w64 (dup'd cols) -> cos/sin
        ph64 = pa.tile([128, 4, PT], F32, tag="avd", name="ph64")
        for ch in range(4):
            for kc in range(2):
                nc.tensor.matmul(out=ph64[:, ch, :],
                                 lhsT=m2v(kc, ch),
                                 rhs=h1_sb[:, kc, :],
                                 start=(kc == 0), stop=(kc == 1))
        csph = const.tile([128, 4, 2, S], F32)
        nc.scalar.activation(out=csph[:, :, 0, :], in_=ph64, func=AF.Sin,
                             bias=pi2_sb)
        nc.scalar.activation(out=csph[:, :, 1, :], in_=ph64, func=AF.Sin)

        # k / k_sw projections with dup'd column layout; issued after the
        # pair-0 q-rope fills so those aren't queued behind 32 cold matmuls
        kproj = {}

        def build_kproj():
            kk = ps.tile([128, 2, 2, PT], F32, tag="sp0", name="kk")
            kkv = kk.rearrange("p a b r -> p (a b) r")
            ks = ps.tile([128, 2, 2, PT], F32, tag="sp1", name="ks")
            ksv = ks.rearrange("p a b r -> p (a b) r")
            for ch in (0, 3, 1, 2):
                for kc in range(4):
                    nc.tensor.matmul(out=kkv[:, ch, :], lhsT=kwv(kc, ch),
                                     rhs=txv[:, kc, :],
                                     start=(kc == 0), stop=(kc == 3))
                for kc in range(4):
                    nc.tensor.matmul(out=ksv[:, ch, :], lhsT=kswv(kc, ch),
                                     rhs=txv[:, kc, :],
                                     start=(kc == 0), stop=(kc == 3))
            kproj['kkv'] = kkv
            kproj['ksv'] = ksv

        tmpa = const.tile([128, 4, S], F32)
        tmpb = const.tile([128, 4, S], F32)
        krot_sb = const.tile([128, 4, S], BF16)
        v_sb = const.tile([128, 2, C], BF16)

        # ---------- main loop helpers ----------

        def qrope_fill(pi, cp, rot_t):
            """q-proj + rope for pair pi, chunk-pair cp -> rot_t[:, 2cp:2cp+2]."""
            gi = pi // PPG
            p0 = (pi % PPG) * PT
            fvst = fvst_tiles[gi]
            qp = pq.tile([128, 2, PT], F32, tag="qp", name="qp")
            for ch2 in range(2):
                ch = 2 * cp + ch2
                for kc in range(2):
                    nc.tensor.matmul(
                        out=qp[:, ch2, :], lhsT=qwv(kc, ch),
                        rhs=fvst[:, kc, p0:p0 + PT],
                        start=(kc == 0), stop=(kc == 1))
            csap = (cs_sb[:, pi * PT:(pi + 1) * PT]
                    .unsqueeze(1).broadcast_to([128, 2, PT]))
            nc.vector.tensor_mul(rot_t[:, 2 * cp:2 * cp + 2, :], qp, csap)

        # Quarter schedule within a pair. Positions pair up (0,1) and (2,3);
        # paired quarters have different `half` (disjoint PE col strips ->
        # 4-way tile concurrency) AND different `g` (disjoint PSUM banks in
        # the g-interleaved avd layout [av-g0, den-g0, av-g1, den-g1]).
        QSEQ = (0, 3, 2, 1)

        def scores(Qp, rot_t):
            p = Qp % 4
            qd = QSEQ[p]
            half, g = qd // 2, qd % 2
            ch = 2 * g + half
            sp = ps.tile([128, 2, 2, PT], F32, tag=f"sp{p % 2}",
                         name="sp", uniquify=True)
            ex = expp.tile([128, 2, 2, PT], BF16, tag="ex",
                           name="ex", uniquify=True)
            for c in range(2):
                for hp2 in range(2):
                    nc.tensor.matmul(
                        out=sp[:, hp2, c, :],
                        lhsT=krot_sb[64 * hp2:64 * hp2 + 64, ch,
                                     c * 128:(c + 1) * 128],
                        rhs=rot_t[64 * hp2:64 * hp2 + 64, ch, :],
                        start=True, stop=True,
                        tile_position=(64 * hp2, 0))
            nc.scalar.activation(out=ex.rearrange("p a b r -> p (a b r)"),
                                 in_=sp.rearrange("p a b r -> p (a b r)"),
                                 func=AF.Exp)
            return ex

        def avden2(avd, exa, qda, exb, qdb):
            """Interleave two quarters' av+den matmuls. The quarters use
            disjoint col strips and disjoint PSUM banks, so the matmuls run
            4-way concurrent while same-bank accumulation groups stay
            sequential."""
            def av_mm(ex, qd, hp2, c):
                half, g = qd // 2, qd % 2
                hp = 2 * half + hp2
                h = 4 * g + hp
                nc.tensor.matmul(out=avd[32 * hp:32 * hp + 32, 2 * g, :],
                                 lhsT=v_sb[:, c, 32 * h:32 * h + 32],
                                 rhs=ex[:, hp2, c, :],
                                 start=(c == 0), stop=(c == 1),
                                 tile_position=(0, 32 * hp))

            def den_mm(ex, qd, hp2, c):
                half, g = qd // 2, qd % 2
                hp = 2 * half + hp2
                nc.tensor.matmul(out=avd[32 * hp:32 * hp + 32, 2 * g + 1, :],
                                 lhsT=ones_sb,
                                 rhs=ex[:, hp2, c, :],
                                 start=(c == 0), stop=(c == 1),
                                 tile_position=(0, 32 * hp))

            for hp2 in range(2):
                for c in range(2):
                    av_mm(exa, qda, hp2, c)
                    av_mm(exb, qdb, hp2, c)
            for hp2 in range(2):
                for c in range(2):
                    den_mm(exa, qda, hp2, c)
                    den_mm(exb, qdb, hp2, c)

        st = {'outst': None}
        HG = 4 * PT  # rows per output-DMA chunk (half group)

        def oproj(pi, adv):
            gi2 = pi // PPG
            if pi % PPG == 0:
                st['outst'] = io.tile([128, 2, GR], BF16, tag="outst",
                                      name="outst")
            outst = st['outst']
            op = po.tile([128, 2, PT], F32, tag="op", name="op")
            for mc in range(2):
                for g in range(2):
                    nc.tensor.matmul(
                        out=op[:, mc, :],
                        lhsT=owv[:, g, mc * 128:(mc + 1) * 128],
                        rhs=adv[:, g, :], start=(g == 0), stop=(g == 1))
            p0 = (pi % PPG) * PT
            nc.vector.tensor_copy(outst[:, :, p0:p0 + PT], op)
            if gi2 == NG - 1:
                for mc in range(2):
                    nc.sync.dma_start(
                        out=out_d[mc * 128:(mc + 1) * 128,
                                  gi2 * GR + p0:gi2 * GR + p0 + PT],
                        in_=outst[:, mc, p0:p0 + PT])
            elif pi % 4 == 3:
                h0 = (pi % PPG - 3) * PT
                for mc in range(2):
                    nc.sync.dma_start(
                        out=out_d[mc * 128:(mc + 1) * 128,
                                  gi2 * GR + h0:gi2 * GR + h0 + HG],
                        in_=outst[:, mc, h0:h0 + HG])

        def recip_div(avd):
            avg = avd.rearrange("p (g s) r -> p g s r", s=2)
            rbc = work.tile([128, 2, PT], F32, tag="rbc", name="rbc")
            nc.vector.reciprocal_approx_fast(rbc, avg[:, :, 1, :])
            adv = work.tile([128, 2, PT], BF16, tag="adiv", name="adiv")
            nc.vector.tensor_mul(adv, avg[:, :, 0, :], rbc)
            return adv

        # krot combine per chunk, in main-loop consumption order.
        def krot_combine():
            kkv, ksv = kproj['kkv'], kproj['ksv']
            for ch in (0, 3, 1, 2):
                nc.vector.tensor_mul(tmpa[:, ch, :], kkv[:, ch, :],
                                     csph[:, ch, 0, :])
                nc.vector.tensor_mul(tmpb[:, ch, :], ksv[:, ch, :],
                                     csph[:, ch, 1, :])
                nc.vector.tensor_add(krot_sb[:, ch, :], tmpa[:, ch, :],
                                     tmpb[:, ch, :])

        # v: [S-chunk, C] bf16 -- first consumed at slot 3; issued after the
        # first scores so its (cold) matmuls don't delay the first exp.
        def build_v():
            for sc in range(2):
                vps = po.tile([128, 2, PT], F32, tag="op", name="vps")
                pv = vps.rearrange("p a r -> p (a r)")[:, 0:C]
                for kc in range(4):
                    nc.tensor.matmul(out=pv,
                                     lhsT=txv[:, kc, sc * 128:(sc + 1) * 128],
                                     rhs=vwv[:, kc, :],
                                     start=(kc == 0), stop=(kc == 3))
                nc.vector.tensor_copy(v_sb[:, sc, :], pv)

        # ---------- main loop: flat quarter pipeline, avden2 at lag 3 ----------

        ex_hist = {}
        avd_by_pair = {}
        adv_by_pair = {}
        rot_by_pair = {}

        def avden_pair(Qa):
            """avden for positions (Qa, Qa+1); Qa % 4 in {0, 2}."""
            P2, pa_pos = divmod(Qa, 4)
            if pa_pos == 0:
                avd_by_pair[P2] = pa.tile([128, 4, PT], F32, tag="avd",
                                          name="avd", uniquify=True)
            avden2(avd_by_pair[P2],
                   ex_hist.pop(Qa), QSEQ[pa_pos],
                   ex_hist.pop(Qa + 1), QSEQ[pa_pos + 1])
            if pa_pos == 2:
                adv_by_pair[P2] = recip_div(avd_by_pair.pop(P2))

        NQ = 4 * NPAIRS
        rot_by_pair[0] = work.tile([128, 4, PT], BF16, tag="rot", name="rot")
        qrope_fill(0, 0, rot_by_pair[0])
        qrope_fill(0, 1, rot_by_pair[0])
        build_kproj()
        krot_combine()

        for Q in range(NQ):
            P, p = divmod(Q, 4)
            gi = P // PPG
            ex_hist[Q] = scores(Q, rot_by_pair[P])
            if Q == 0:
                build_v()
            if p == 0 and P + 1 < NPAIRS:
                rot_by_pair[P + 1] = work.tile([128, 4, PT], BF16, tag="rot",
                                               name="rot", uniquify=True)
                qrope_fill(P + 1, 0, rot_by_pair[P + 1])
            if p == 1 and P + 1 < NPAIRS:
                qrope_fill(P + 1, 1, rot_by_pair[P + 1])
            if p % 2 == 1 and Q >= 3:
                avden_pair(Q - 3)
            if p == 2 and P - 1 in adv_by_pair:
                oproj(P - 1, adv_by_pair.pop(P - 1))
            if p == 3:
                rot_by_pair.pop(P)
                if P % PPG == 0 and gi + 1 < NG:
                    load_group(gi + 1)

        # ---------- tail ----------
        avden_pair(NQ - 2)
        oproj(NPAIRS - 1, adv_by_pair.pop(NPAIRS - 1))


_NC_CACHE = {}


def _get_nc():
    if 'nc' not in _NC_CACHE:
        _NC_CACHE['nc'] = build_nc()
    return _NC_CACHE['nc']


def _run(inputs, trace=False):
    from concourse.bass_utils import run_bass_kernel_spmd
    nc = _get_nc()
    in_maps = host_prep(inputs)
    res = run_bass_kernel_spmd(nc, in_maps, core_ids=list(range(N_CORES)),
                               trace=trace)
    return gather_out(res.results), res


def kernel(**inputs):
    out, _ = _run(inputs, trace=False)
    return out


# revision 35
# speedup vs baseline: 1.0885x; 1.0020x over previous
"""Trainium2 Bass kernel for nn_AnatomicalSpaceAttention_5952824672905.

Self-contained: kernel(**inputs) takes the FULL unsharded inputs, shards
across 8 NeuronCores (core c -> batch c//4, D-planes [8*(c%4), 8*(c%4)+8)),
runs one SPMD Bass graph (no collectives -- cross-attention rows are
independent), and gathers the full [B, C, D, H, W] f32 output.

Structure (per core, 8192 query rows x 8 heads x 256 keys):
  - q-projection emits 64-dim "rope features" per head directly:
    weight columns are host-reordered to [h_even: q(32)|q_sw(32),
    h_odd: q(32)|q_sw(32)] per 128-chunk, so a single DVE multiply by a
    fused cos/sin table produces rot64 = [q*cos | q_sw*sin]; scores
    contract K=64 against krot duplicated twice -- no DVE add needed.
  - flat quarter pipeline: scores(Q)+exp(Q) -> avden(Q-1), uniform
    across pair boundaries so ScalarE (exp) never bubbles.
"""

import sys

for _p in ('/opt/trn_rl_repo', '/root/.axon_site/_ro/trn_rl_repo'):
    if _p not in sys.path:
        sys.path.append(_p)

import numpy as np
import ml_dtypes

import concourse.bass as bass
import concourse.mybir as mybir
import concourse.tile as tile
from concourse import bacc

BF16 = mybir.dt.bfloat16
F32 = mybir.dt.float32
AF = mybir.ActivationFunctionType

NH, HD = 8, 32
C, S, TD = 256, 256, 512
D = H = W = 32
N_CORES = 8
DSL = D // 4            # 8 d-planes per core
ROWS = DSL * H * W      # 8192
PT = 256                # rows per pair
NPAIRS = ROWS // PT     # 32
GR = 2048               # rows per DMA group
NG = ROWS // GR         # 4
PPG = GR // PT          # 8 pairs per group
SCALE = float(HD) ** -0.5
BASE = 10000.0

bf16 = ml_dtypes.bfloat16

# wpack block offsets (each block is [128, 128] bf16). Order = DMA
# priority: text-side weights first (they gate the serial prologue chain).
M1_O, TX_O, M2_O, KW_O, KSW_O, QW_O, VW_O, OW_O, NBLK = \
    0, 8, 16, 24, 40, 56, 64, 72, 76


# ----------------------------------------------------------------- host prep

def _inv_freq(dim):
    return 1.0 / (BASE ** (np.arange(0, dim, 2, dtype=np.float64) / dim))


def rope_freqs_full():
    """[D, H, W, HD] f64 -- matches reference.rope3d_freqs."""
    zd = HD // 3
    yd = HD // 3
    xd = HD - zd - yd
    fz = np.arange(D, dtype=np.float64)[:, None] * _inv_freq(zd)
    fy = np.arange(H, dtype=np.float64)[:, None] * _inv_freq(yd)
    fx = np.arange(W, dtype=np.float64)[:, None] * _inv_freq(xd)
    ez = np.broadcast_to(np.concatenate([fz, fz], -1)[:, None, None, :], (D, H, W, zd))
    ey = np.broadcast_to(np.concatenate([fy, fy], -1)[None, :, None, :], (D, H, W, yd))
    ex = np.broadcast_to(np.concatenate([fx, fx], -1)[None, None, :, :], (D, H, W, xd))
    return np.concatenate([ez, ey, ex], axis=-1)  # [D,H,W,HD]


def swap_w(wm):
    """Column-permute+negate so x @ w_sw == rotate_half(x @ w) per 32-head-dim."""
    w = np.asarray(wm)
    out = np.empty_like(w)
    for h in range(NH):
        blk = w[:, h * HD:(h + 1) * HD]
        out[:, h * HD:h * HD + 16] = -blk[:, 16:32]
        out[:, h * HD + 16:(h + 1) * HD] = blk[:, 0:16]
    return out


def pack64(wa, wb):
    """[K,256] x2 -> [K,512]; col ch*128 + a*64 + d*32 + j takes
    (wa if d==0 else wb)[:, 32*(2ch+a)+j]."""
    K = wa.shape[0]
    out = np.empty((K, 512), np.float64)
    o = out.reshape(K, 4, 2, 2, 32)
    A = wa.reshape(K, NH, 32)
    B = wb.reshape(K, NH, 32)
    for ch in range(4):
        for a in range(2):
            o[:, ch, a, 0, :] = A[:, 2 * ch + a, :]
            o[:, ch, a, 1, :] = B[:, 2 * ch + a, :]
    return out


def host_prep(inputs):
    fv = np.asarray(inputs['fused_visual'], dtype=np.float32)   # [B,C,D,H,W]
    te = np.asarray(inputs['text_embedding'], dtype=np.float32)  # [B,S,TD]
    q_w = np.asarray(inputs['q_w'], dtype=np.float64)
    k_w = np.asarray(inputs['k_w'], dtype=np.float64)
    v_w = np.asarray(inputs['v_w'], dtype=np.float64)
    o_w = np.asarray(inputs['o_w'], dtype=np.float64)
    m1_w = np.asarray(inputs['m1_w'], dtype=np.float64)
    m2_w = np.asarray(inputs['m2_w'], dtype=np.float64)

    qw64 = pack64(q_w * SCALE, swap_w(q_w) * SCALE)   # [256, 512]
    kw64 = pack64(k_w, k_w)                            # [512, 512]
    ksw64 = pack64(swap_w(k_w), swap_w(k_w))           # [512, 512]
    m2w64 = pack64(m2_w, m2_w)                         # [256, 512]

    freqs = rope_freqs_full()
    cosf = np.cos(freqs)
    sinf = np.sin(freqs)

    blocks = []

    def add64(w):
        for kc in range(w.shape[0] // 128):
            for ch in range(4):
                blocks.append(w[kc * 128:(kc + 1) * 128, ch * 128:(ch + 1) * 128])

    def add256(w):
        for kc in range(w.shape[0] // 128):
            blocks.append(w[kc * 128:(kc + 1) * 128, 0:128])
            blocks.append(w[kc * 128:(kc + 1) * 128, 128:256])

    in_maps = []
    for core in range(N_CORES):
        b = core // 4
        g4 = core % 4
        dsl = slice(g4 * DSL, (g4 + 1) * DSL)

        fv_sh = np.ascontiguousarray(
            fv[b, :, dsl].reshape(C, ROWS)).astype(bf16)

        cT = cosf[dsl].reshape(ROWS, HD).T   # [32, ROWS]
        sT = sinf[dsl].reshape(ROWS, HD).T
        cs64 = np.ascontiguousarray(
            np.concatenate([cT, sT, cT, sT], axis=0)).astype(bf16)  # [128, ROWS]

        textT = te[b].T.astype(np.float64)   # [TD, S]
        blocks.clear()
        add256(m1_w)
        add256(textT)
        add64(m2w64)
        add64(kw64)
        add64(ksw64)
        add64(qw64)
        add256(v_w)
        add256(o_w)
        wblob = np.ascontiguousarray(
            np.concatenate(blocks, axis=1)).astype(bf16)  # [128, 76*128]

        in_maps.append({'fv': fv_sh, 'cs': cs64, 'wpack': wblob})
    return in_maps


def gather_out(results):
    """Per-core [C, ROWS] bf16 -> full [B, C, D, H, W] f32."""
    B = 2
    out = np.empty((B, C, D, H, W), dtype=np.float32)
    for core in range(N_CORES):
        b = core // 4
        g4 = core % 4
        out[b, :, g4 * DSL:(g4 + 1) * DSL] = (
            np.asarray(results[core]['out']).astype(np.float32)
            .reshape(C, DSL, H, W))
    return out


# ------------------------------------------------------------------- builder

def build_nc():
    nc = bacc.Bacc("TRN2", target_bir_lowering=False, debug=False)

    fv_d = nc.dram_tensor("fv", [C, ROWS], BF16, kind="ExternalInput")
    cs_d = nc.dram_tensor("cs", [128, ROWS], BF16, kind="ExternalInput")
    wpack_d = nc.dram_tensor("wpack", [128, NBLK * 128], BF16,
                             kind="ExternalInput")
    out_d = nc.dram_tensor("out", [C, ROWS], BF16, kind="ExternalOutput")

    with tile.TileContext(nc) as tc:
        _graph(tc, nc, fv_d, cs_d, wpack_d, out_d)

    nc.compile()
    return nc


def _graph(tc, nc, fv_d, cs_d, wpack_d, out_d):
    from contextlib import ExitStack
    ctx = ExitStack()
    with ctx:
        const = ctx.enter_context(tc.tile_pool(name="const", bufs=1))
        io = ctx.enter_context(tc.tile_pool(name="io", bufs=2))
        work = ctx.enter_context(tc.tile_pool(name="work", bufs=2))
        expp = ctx.enter_context(tc.tile_pool(name="expp", bufs=6))
        pq = ctx.enter_context(tc.tile_pool(name="pq", bufs=1, space="PSUM"))
        ps = ctx.enter_context(tc.tile_pool(name="ps", bufs=1, space="PSUM"))
        pa = ctx.enter_context(tc.tile_pool(name="pa", bufs=1, space="PSUM"))
        po = ctx.enter_context(tc.tile_pool(name="po", bufs=1, space="PSUM"))

        # ---------- PE warm-up burst (opens HAM clock gate) ----------
        wub = const.tile([1, 512], BF16)
        nc.vector.memset(wub, 0.0)
        wu = ps.tile([128, 2, 2, PT], F32, tag="sp0", name="wu")
        wuf = wu.rearrange("p a b r -> p (a b r)")
        for _ in range(7):
            nc.tensor.matmul(out=wuf[:, 0:512], lhsT=wub[0:1, 0:128],
                             rhs=wub, start=True, stop=True)

        # ---------- DMAs ----------
        # Split by prologue criticality: text-side weights gate the serial
        # krot chain; qw/fv/cs gate pair-0 q-proj; vw/ow are needed later.
        wpack_sb = const.tile([128, NBLK, 128], BF16)

        def wdma(b0, b1):
            nc.sync.dma_start(
                out=wpack_sb[:, b0:b1].rearrange("p a b -> p (a b)"),
                in_=wpack_d[:, b0 * 128:b1 * 128])

        wdma(M1_O, M2_O)           # m1 + textT
        wdma(M2_O, M2_O + 8)       # m2w64
        wdma(KW_O, KSW_O + 16)     # kw64 + ksw64
        wdma(QW_O, QW_O + 8)       # qw64

        cs_sb = const.tile([128, ROWS], BF16)
        fvst_tiles = {}

        def load_group(gi):
            t = io.tile([128, 2, GR], BF16, tag="fvst", name="fvst")
            for kc in range(2):
                nc.sync.dma_start(
                    out=t[:, kc, :],
                    in_=fv_d[kc * 128:(kc + 1) * 128, gi * GR:(gi + 1) * GR])
            nc.sync.dma_start(out=cs_sb[:, gi * GR:(gi + 1) * GR],
                              in_=cs_d[:, gi * GR:(gi + 1) * GR])
            fvst_tiles[gi] = t

        load_group(0)
        wdma(VW_O, NBLK)           # vw + ow

        # ---------- constants ----------
        ones_sb = const.tile([128, HD], BF16)
        nc.vector.memset(ones_sb, 1.0)
        pi2_sb = const.tile([128, 1], F32)
        nc.vector.memset(pi2_sb, float(np.pi / 2))

        def wchunk(idx):
            return wpack_sb[:, idx, :]

        def qwv(kc, ch):
            return wchunk(QW_O + kc * 4 + ch)

        def kwv(kc, ch):
            return wchunk(KW_O + kc * 4 + ch)

        def kswv(kc, ch):
            return wchunk(KSW_O + kc * 4 + ch)

        def m2v(kc, ch):
            return wchunk(M2_O + kc * 4 + ch)

        vwv = wpack_sb[:, VW_O:VW_O + 8].rearrange("p (a b) f -> p a (b f)", b=2)
        m1v = wpack_sb[:, M1_O:M1_O + 8].rearrange("p (a b) f -> p a (b f)", b=2)
        owv = wpack_sb[:, OW_O:OW_O + 4].rearrange("p (a b) f -> p a (b f)", b=2)
        txv = wpack_sb[:, TX_O:TX_O + 8].rearrange("p (a b) f -> p a (b f)", b=2)

        # ---------- text side (once) ----------
        # h1 = gelu(text @ m1_w): [128, 2(mc), S] bf16
        h1_sb = const.tile([128, 2, S], BF16)
        for mc in range(2):
            pool = pq if mc == 0 else po
            h1ps = pool.tile([128, 2, PT], F32,
                             tag="qp" if mc == 0 else "op", name="h1ps")
            pv = h1ps.rearrange("p a r -> p (a r)")[:, 0:S]
            for kc in range(4):
                nc.tensor.matmul(out=pv,
                                 lhsT=m1v[:, kc, mc * 128:(mc + 1) * 128],
                                 rhs=txv[:, kc, :],
                                 start=(kc == 0), stop=(kc == 3))
            nc.scalar.activation(out=h1_sb[:, mc, :], in_=pv, func=AF.Gelu)

        # phase64 = h1 @ m2w64 (dup'd cols) -> cos/sin
        ph64 = pa.tile([128, 4, PT], F32, tag="avd", name="ph64")
        for ch in range(4):
            for kc in range(2):
                nc.tensor.matmul(out=ph64[:, ch, :],
                                 lhsT=m2v(kc, ch),
                                 rhs=h1_sb[:, kc, :],
                                 start=(kc == 0), stop=(kc == 1))
        csph = const.tile([128, 4, 2, S], F32)
        nc.scalar.activation(out=csph[:, :, 0, :], in_=ph64, func=AF.Sin,
                             bias=pi2_sb)
        nc.scalar.activation(out=csph[:, :, 1, :], in_=ph64, func=AF.Sin)

        # k / k_sw projections with dup'd column layout; issued after the
        # pair-0 q-rope fills so those aren't queued behind 32 cold matmuls
        kproj = {}

        def build_kproj():
            kk = ps.tile([128, 2, 2, PT], F32, tag="sp0", name="kk")
            kkv = kk.rearrange("p a b r -> p (a b) r")
            ks = ps.tile([128, 2, 2, PT], F32, tag="sp1", name="ks")
            ksv = ks.rearrange("p a b r -> p (a b) r")
            for ch in (0, 3, 1, 2):
                for kc in range(4):
                    nc.tensor.matmul(out=kkv[:, ch, :], lhsT=kwv(kc, ch),
                                     rhs=txv[:, kc, :],
                                     start=(kc == 0), stop=(kc == 3))
                for kc in range(4):
                    nc.tensor.matmul(out=ksv[:, ch, :], lhsT=kswv(kc, ch),
                                     rhs=txv[:, kc, :],
                                     start=(kc == 0), stop=(kc == 3))
            kproj['kkv'] = kkv
            kproj['ksv'] = ksv

        tmpa = const.tile([128, 4, S], F32)
        tmpb = const.tile([128, 4, S], F32)
        krot_sb = const.tile([128, 4, S], BF16)
        v_sb = const.tile([128, 2, C], BF16)

        # ---------- main loop helpers ----------

        def qrope_fill(pi, cp, rot_t):
            """q-proj + rope for pair pi, chunk-pair cp -> rot_t[:, 2cp:2cp+2]."""
            gi = pi // PPG
            p0 = (pi % PPG) * PT
            fvst = fvst_tiles[gi]
            qp = pq.tile([128, 2, PT], F32, tag="qp", name="qp")
            for ch2 in range(2):
                ch = 2 * cp + ch2
                for kc in range(2):
                    nc.tensor.matmul(
                        out=qp[:, ch2, :], lhsT=qwv(kc, ch),
                        rhs=fvst[:, kc, p0:p0 + PT],
                        start=(kc == 0), stop=(kc == 1))
            csap = (cs_sb[:, pi * PT:(pi + 1) * PT]
                    .unsqueeze(1).broadcast_to([128, 2, PT]))
            nc.vector.tensor_mul(rot_t[:, 2 * cp:2 * cp + 2, :], qp, csap)

        # Quarter schedule within a pair. Positions pair up (0,1) and (2,3);
        # paired quarters have different `half` (disjoint PE col strips ->
        # 4-way tile concurrency) AND different `g` (disjoint PSUM banks in
        # the g-interleaved avd layout [av-g0, den-g0, av-g1, den-g1]).
        QSEQ = (0, 3, 2, 1)

        def scores(Qp, rot_t):
            p = Qp % 4
            qd = QSEQ[p]
            half, g = qd // 2, qd % 2
            ch = 2 * g + half
            sp = ps.tile([128, 2, 2, PT], F32, tag=f"sp{p % 2}",
                         name="sp", uniquify=True)
            ex = expp.tile([128, 2, 2, PT], BF16, tag="ex",
                           name="ex", uniquify=True)
            for c in range(2):
                for hp2 in range(2):
                    nc.tensor.matmul(
                        out=sp[:, hp2, c, :],
                        lhsT=krot_sb[64 * hp2:64 * hp2 + 64, ch,
                                     c * 128:(c + 1) * 128],
                        rhs=rot_t[64 * hp2:64 * hp2 + 64, ch, :],
                        start=True, stop=True,
                        tile_position=(64 * hp2, 0))
            nc.scalar.activation(out=ex.rearrange("p a b r -> p (a b r)"),
                                 in_=sp.rearrange("p a b r -> p (a b r)"),
                                 func=AF.Exp)
            return ex

        def avden2(avd, exa, qda, exb, qdb):
            """Interleave two quarters' av+den matmuls. The quarters use
            disjoint col strips and disjoint PSUM banks, so the matmuls run
            4-way concurrent while same-bank accumulation groups stay
            sequential."""
            def av_mm(ex, qd, hp2, c):
                half, g = qd // 2, qd % 2
                hp = 2 * half + hp2
                h = 4 * g + hp
                nc.tensor.matmul(out=avd[32 * hp:32 * hp + 32, 2 * g, :],
                                 lhsT=v_sb[:, c, 32 * h:32 * h + 32],
                                 rhs=ex[:, hp2, c, :],
                                 start=(c == 0), stop=(c == 1),
                                 tile_position=(0, 32 * hp))

            def den_mm(ex, qd, hp2, c):
                half, g = qd // 2, qd % 2
                hp = 2 * half + hp2
                nc.tensor.matmul(out=avd[32 * hp:32 * hp + 32, 2 * g + 1, :],
                                 lhsT=ones_sb,
                                 rhs=ex[:, hp2, c, :],
                                 start=(c == 0), stop=(c == 1),
                                 tile_position=(0, 32 * hp))

            for hp2 in range(2):
                for c in range(2):
                    av_mm(exa, qda, hp2, c)
                    av_mm(exb, qdb, hp2, c)
            for hp2 in range(2):
                for c in range(2):
                    den_mm(exa, qda, hp2, c)
                    den_mm(exb, qdb, hp2, c)

        st = {'outst': None}
        HG = 4 * PT  # rows per output-DMA chunk (half group)

        def oproj(pi, adv):
            gi2 = pi // PPG
            if pi % PPG == 0:
                st['outst'] = io.tile([128, 2, GR], BF16, tag="outst",
                                      name="outst")
            outst = st['outst']
            op = po.tile([128, 2, PT], F32, tag="op", name="op")
            for mc in range(2):
                for g in range(2):
                    nc.tensor.matmul(
                        out=op[:, mc, :],
                        lhsT=owv[:, g, mc * 128:(mc + 1) * 128],
                        rhs=adv[:, g, :], start=(g == 0), stop=(g == 1))
            p0 = (pi % PPG) * PT
            nc.vector.tensor_copy(outst[:, :, p0:p0 + PT], op)
            if gi2 == NG - 1:
                for mc in range(2):
                    nc.sync.dma_start(
                        out=out_d[mc * 128:(mc + 1) * 128,
                                  gi2 * GR + p0:gi2 * GR + p0 + PT],
                        in_=outst[:, mc, p0:p0 + PT])
            elif pi % 4 == 3:
                h0 = (pi % PPG - 3) * PT
                for mc in range(2):
                    nc.sync.dma_start(
                        out=out_d[mc * 128:(mc + 1) * 128,
                                  gi2 * GR + h0:gi2 * GR + h0 + HG],
                        in_=outst[:, mc, h0:h0 + HG])

        def recip_div(avd):
            avg = avd.rearrange("p (g s) r -> p g s r", s=2)
            rbc = work.tile([128, 2, PT], F32, tag="rbc", name="rbc")
            nc.vector.reciprocal_approx_fast(rbc, avg[:, :, 1, :])
            adv = work.tile([128, 2, PT], BF16, tag="adiv", name="adiv")
            nc.vector.tensor_mul(adv, avg[:, :, 0, :], rbc)
            return adv

        # krot combine per chunk, in main-loop consumption order.
        def krot_combine():
            kkv, ksv = kproj['kkv'], kproj['ksv']
            for ch in (0, 3, 1, 2):
                nc.vector.tensor_mul(tmpa[:, ch, :], kkv[:, ch, :],
                                     csph[:, ch, 0, :])
                nc.vector.tensor_mul(tmpb[:, ch, :], ksv[:, ch, :],
                                     csph[:, ch, 1, :])
                nc.vector.tensor_add(krot_sb[:, ch, :], tmpa[:, ch, :],
                                     tmpb[:, ch, :])

        # v: [S-chunk, C] bf16 -- first consumed at slot 3; issued after the
        # first scores so its (cold) matmuls don't delay the first exp.
        def build_v():
            for sc in range(2):
                vps = po.tile([128, 2, PT], F32, tag="op", name="vps")
                pv = vps.rearrange("p a r -> p (a r)")[:, 0:C]
                for kc in range(4):
                    nc.tensor.matmul(out=pv,
                                     lhsT=txv[:, kc, sc * 128:(sc + 1) * 128],
                                     rhs=vwv[:, kc, :],
                                     start=(kc == 0), stop=(kc == 3))
                nc.vector.tensor_copy(v_sb[:, sc, :], pv)

        # ---------- main loop: flat quarter pipeline, avden2 at lag 3 ----------

        ex_hist = {}
        avd_by_pair = {}
        adv_by_pair = {}
        rot_by_pair = {}

        def avden_pair(Qa):
            """avden for positions (Qa, Qa+1); Qa % 4 in {0, 2}."""
            P2, pa_pos = divmod(Qa, 4)
            if pa_pos == 0:
                avd_by_pair[P2] = pa.tile([128, 4, PT], F32, tag="avd",
                                          name="avd", uniquify=True)
            avden2(avd_by_pair[P2],
                   ex_hist.pop(Qa), QSEQ[pa_pos],
                   ex_hist.pop(Qa + 1), QSEQ[pa_pos + 1])
            if pa_pos == 2:
                adv_by_pair[P2] = recip_div(avd_by_pair.pop(P2))

        NQ = 4 * NPAIRS
        rot_by_pair[0] = work.tile([128, 4, PT], BF16, tag="rot", name="rot")
        qrope_fill(0, 0, rot_by_pair[0])
        qrope_fill(0, 1, rot_by_pair[0])
        build_kproj()
        krot_combine()

        for Q in range(NQ):
            P, p = divmod(Q, 4)
            gi = P // PPG
            ex_hist[Q] = scores(Q, rot_by_pair[P])
            if Q == 0:
                build_v()
            if p == 0 and P + 1 < NPAIRS:
                rot_by_pair[P + 1] = work.tile([128, 4, PT], BF16, tag="rot",
                                               name="rot", uniquify=True)
                qrope_fill(P + 1, 0, rot_by_pair[P + 1])
            if p == 1 and P + 1 < NPAIRS:
                qrope_fill(P + 1, 1, rot_by_pair[P + 1])
            if p == 0 and Q >= 4:
                avden_pair(Q - 4)
            if p == 1 and Q >= 3:
                avden_pair(Q - 3)
            if p == 2 and P - 1 in adv_by_pair:
                oproj(P - 1, adv_by_pair.pop(P - 1))
            if p == 3:
                rot_by_pair.pop(P)
                if P % PPG == 0 and gi + 1 < NG:
                    load_group(gi + 1)

        # ---------- tail ----------
        avden_pair(NQ - 4)
        avden_pair(NQ - 2)
        oproj(NPAIRS - 1, adv_by_pair.pop(NPAIRS - 1))


_NC_CACHE = {}


def _get_nc():
    if 'nc' not in _NC_CACHE:
        _NC_CACHE['nc'] = build_nc()
    return _NC_CACHE['nc']


def _run(inputs, trace=False):
    from concourse.bass_utils import run_bass_kernel_spmd
    nc = _get_nc()
    in_maps = host_prep(inputs)
    res = run_bass_kernel_spmd(nc, in_maps, core_ids=list(range(N_CORES)),
                               trace=trace)
    return gather_out(res.results), res


def kernel(**inputs):
    out, _ = _run(inputs, trace=False)
    return out
